# revision 3
# baseline (speedup 1.0000x reference)
"""Causal self-attention (B=4, T=2048, D=1024, H=16) on 8 trn2 NeuronCores.

Sharding: 2-D data x tensor parallel. Core c handles batch b = c//2 and
head group hg = c%2 (8 of the 16 heads). Each core computes its 8 heads'
qkv projection, causal attention, and a partial output projection
(columns of w_out for its heads); the host sums the two partials per
batch element and adds b_out.

Precision/layout strategy (validated vs the fp32 reference, ~1e-2 max rel
err against a 2e-2 budget):
  - q/k projection in fp8e4 DoubleRow (k-dim pairs packed 2/partition):
    x scaled x32, w_qk x2048, all scales powers of 2 so the net factor
    folds exactly into the exp() scale (2^-13) and the bias (x32).
  - q/k stored fp8 [128, block, slot(2), T]; slot 1 is zeros so the
    S = K^T Q matmul can also run DoubleRow (contraction 64 = 64 real +
    64 zero) at the fp8 streaming rate.
  - v path and out-proj stay bf16: they feed the output directly.
  - exp on ACT with causal narrowing; P bf16; PV via v_aug [tok, 65]
    (65th col ones => PSUM row 64 = softmax denominator).
  - diagonal S/PV matmuls narrowed to the live query range, so the
    below-diagonal et region is never read (no memsets in the hot loop).
  - S(kt+1) is emitted before exp(kt)/PV(kt): PE streams the next score
    tile while ACT exponentiates the current one.
"""
import sys

import numpy as np

if "/opt/trn_rl_repo" not in sys.path:
    sys.path.insert(0, "/opt/trn_rl_repo")

import ml_dtypes

D = 1024          # d_model
T = 2048          # seq len
B = 4             # batch
HD = 64           # head dim
KT = 8            # d_model k-tiles of 128
NTT = 16          # token tiles of 128
NTB = 4           # token blocks of 512
NPAIR = 4         # head pairs per core (8 heads)
VSTR = 8 * 65     # v_aug cols per token tile (8 heads x 65)
WARMUP_MM = 32

_CACHE = {}


def _build_program():
    import concourse.mybir as mybir
    import concourse.tile as tile
    from concourse import bacc

    dt = mybir.dt
    f32, bf16, f8 = dt.float32, dt.bfloat16, dt.float8e4
    AF = mybir.ActivationFunctionType
    DR = mybir.MatmulPerfMode.DoubleRow

    nc = bacc.Bacc("TRN2", target_bir_lowering=False, debug=False,
                   enable_asserts=False, num_devices=8)

    xf8_d = nc.dram_tensor("xf8", [128, KT * T], f8, kind="ExternalInput").ap()
    x16_d = nc.dram_tensor("x16", [128, KT * T], bf16, kind="ExternalInput").ap()
    wqk8_d = nc.dram_tensor("wqk8", [128, 8192], f8, kind="ExternalInput").ap()
    wv16_d = nc.dram_tensor("wv16", [128, KT * 512], bf16, kind="ExternalInput").ap()
    bqk_d = nc.dram_tensor("bqk", [128, 8], f32, kind="ExternalInput").ap()
    bv_d = nc.dram_tensor("bv", [128, 512], f32, kind="ExternalInput").ap()
    zf8_d = nc.dram_tensor("zf8", [128, T], f8, kind="ExternalInput").ap()
    woT_d = nc.dram_tensor("woT", [512, 1024], bf16, kind="ExternalInput").ap()
    mask_d = nc.dram_tensor("mask2", [128, 256], bf16, kind="ExternalInput").ap()
    outT_d = nc.dram_tensor("outT", [D, T], f32, kind="ExternalOutput").ap()
    warm_d = nc.dram_tensor("warm", [1, 512], f32, kind="ExternalOutput").ap()

    with tile.TileContext(nc) as tc:
        with tc.tile_pool(name="const", bufs=1) as cpool, \
             tc.tile_pool(name="qk8", bufs=1) as qkpool, \
             tc.tile_pool(name="xt", bufs=1) as xpool, \
             tc.tile_pool(name="vt", bufs=1) as vpool, \
             tc.tile_pool(name="exp", bufs=3) as epool, \
             tc.tile_pool(name="at", bufs=1) as apool, \
             tc.tile_pool(name="rcp", bufs=2) as rpool, \
             tc.tile_pool(name="rbc", bufs=2) as rbpool, \
             tc.tile_pool(name="stg", bufs=2) as spool, \
             tc.tile_pool(name="big", bufs=2, space="PSUM") as pp_big, \
             tc.tile_pool(name="pv", bufs=2, space="PSUM") as pp_pv:

            # ---- PE warm-up: keep the clock un-throttled during DMA ----
            wtile = cpool.tile([128, 512], bf16, tag="wrm")
            nc.vector.memset(wtile[:], 0.001)
            wps = pp_big.tile([128, 1024], f32, tag="big")
            for i in range(WARMUP_MM):
                nc.tensor.matmul(wps[:, 0:512], wtile[:, 0:128], wtile[:],
                                 start=(i == 0), stop=(i == WARMUP_MM - 1))
            wout = cpool.tile([1, 512], f32, tag="wout")
            nc.vector.tensor_copy(wout[:], wps[0:1, 0:512])
            nc.sync.dma_start(warm_d, wout[:])

            # ---- constant loads (ordered: first-needed first) ----
            bqk_sb = cpool.tile([128, 8], f32, tag="bqk")
            nc.sync.dma_start(bqk_sb[:], bqk_d)
            m_order = [0, 4, 1, 5, 2, 6, 3, 7]
            wqk8_sb = cpool.tile([128, 8192], f8, tag="wqk8")
            for m in m_order:
                nc.sync.dma_start(wqk8_sb[:, m * 1024:(m + 1) * 1024],
                                  wqk8_d[:, m * 1024:(m + 1) * 1024])
            xf8_sb = xpool.tile([128, KT * T], f8, tag="xf8")
            for half in range(2):
                for kt in range(KT):
                    nc.sync.dma_start(
                        xf8_sb[:, kt * T + half * 1024: kt * T + (half + 1) * 1024],
                        xf8_d[:, kt * T + half * 1024: kt * T + (half + 1) * 1024])
            # q/k fp8 store: [block m (8), slot (2), T]; slot 1 = zeros
            qk8_sb = qkpool.tile([128, 8 * 2 * T], f8, tag="qk8")
            for m in range(8):
                nc.sync.dma_start(
                    qk8_sb[:, (2 * m + 1) * T: (2 * m + 2) * T], zf8_d)
            wv16_sb = cpool.tile([128, KT * 512], bf16, tag="wv")
            nc.sync.dma_start(wv16_sb[:], wv16_d)
            mask_sb = cpool.tile([128, 256], bf16, tag="mask")
            nc.sync.dma_start(mask_sb[:], mask_d)
            bv_sb = cpool.tile([128, 512], f32, tag="bv")
            nc.sync.dma_start(bv_sb[:], bv_d)
            # x bf16 chunked [kt, nb], nb-major so early token blocks land first
            x16_sb = xpool.tile([128, KT * T], bf16, tag="x16")
            for nb in range(NTB):
                for kt in range(KT):
                    nc.sync.dma_start(
                        x16_sb[:, kt * T + nb * 512: kt * T + (nb + 1) * 512],
                        x16_d[:, kt * T + nb * 512: kt * T + (nb + 1) * 512])
            woT_sb = cpool.tile([128, 4 * 1024], bf16, tag="wo")
            nc.sync.dma_start(
                woT_sb[:].rearrange("p (k f) -> p k f", k=4),
                woT_d.rearrange("(k p) f -> p k f", p=128))

            # ---- qk projection: fp8 DoubleRow, all 8 feature blocks ----
            xf8_v = xf8_sb[:].rearrange("p (k t) -> p k t", k=KT)
            for m in m_order:
                for nb2 in range(2):
                    ps = pp_big.tile([128, 1024], f32, tag="big")
                    for ktp in range(4):
                        lhsT = wqk8_sb[:, (m * 4 + ktp) * 256:
                                       (m * 4 + ktp) * 256 + 256] \
                            .rearrange("p (j f) -> p j f", j=2)
                        for u in range(2):
                            c0 = nb2 * 1024 + u * 512
                            nc.tensor.matmul(
                                ps[:, u * 512:(u + 1) * 512],
                                lhsT,
                                xf8_v[:, 2 * ktp:2 * ktp + 2, c0:c0 + 512],
                                start=(ktp == 0), stop=(ktp == 3),
                                perf_mode=DR)
                    nc.scalar.activation(
                        qk8_sb[:, 2 * m * T + nb2 * 1024:
                               2 * m * T + nb2 * 1024 + 1024],
                        ps[:], AF.Identity,
                        bias=bqk_sb[:, m:m + 1], scale=float(2.0 ** -11))

            # ---- v phase: v_aug [token, head*65] bf16 (65th col = ones) ----
            v_sb = vpool.tile([128, NTT * VSTR], bf16)
            nc.vector.memset(
                v_sb[:].rearrange("p (g e) -> p g e", e=65)[:, :, 64:65], 1.0)
            for tt2 in range(NTT // 2):
                ps = pp_big.tile([128, 1024], f32, tag="big")
                for kt in range(KT):
                    for u in range(2):
                        tt = 2 * tt2 + u
                        nc.tensor.matmul(
                            ps[:, u * 512:(u + 1) * 512],
                            x16_sb[:, kt * T + tt * 128: kt * T + (tt + 1) * 128],
                            wv16_sb[:, kt * 512:(kt + 1) * 512],
                            start=(kt == 0), stop=(kt == KT - 1))
                nc.vector.tensor_add(
                    v_sb[:].rearrange("p (t h e) -> p t h e", t=NTT, h=8)
                        [:, 2 * tt2:2 * tt2 + 2, :, 0:64],
                    ps[:].rearrange("p (u h f) -> p u h f", u=2, h=8),
                    bv_sb[:].rearrange("p (u h f) -> p u h f", u=1, h=8)
                         .to_broadcast([128, 2, 8, 64]))

            # ---- per head-pair attention ----
            attn_sb = apool.tile([128, NPAIR * T], bf16)

            def emit_outproj(tbs, mds):
                for md in mds:
                    ps = pp_big.tile([128, 1024], f32, tag="big")
                    for kf in range(4):
                        for u, tb in enumerate(tbs):
                            nc.tensor.matmul(
                                ps[:, u * 512:(u + 1) * 512],
                                woT_sb[:, kf * 1024 + md * 128:
                                       kf * 1024 + (md + 1) * 128],
                                attn_sb[:, kf * T + tb * 512:
                                        kf * T + tb * 512 + 512],
                                start=(kf == 0), stop=(kf == 3))
                    st = spool.tile([128, 1024], f32, tag="stg")
                    w = 512 * len(tbs)
                    nc.vector.tensor_copy(st[0:128, 0:w], ps[0:128, 0:w])
                    nc.sync.dma_start(
                        outT_d[md * 128:(md + 1) * 128,
                               tbs[0] * 512: tbs[0] * 512 + w], st[0:128, 0:w])

            for pair in range(NPAIR):
                qz = qk8_sb[:, 2 * pair * T: 2 * pair * T + 2 * T] \
                    .rearrange("p (j t) -> p j t", j=2)
                kz = qk8_sb[:, 2 * (4 + pair) * T: 2 * (4 + pair) * T + 2 * T] \
                    .rearrange("p (j t) -> p j t", j=2)

                for qb in range(NTB):
                    nkt = 4 * qb + 4
                    sc_tiles = {}

                    def emit_s(kt, qb=qb, qz=qz, kz=kz, sc_tiles=sc_tiles):
                        off = max(kt - 4 * qb, 0) * 128
                        sc = pp_big.tile([128, 1024], f32, tag="big")
                        for hh in range(2):
                            nc.tensor.matmul(
                                sc[:, hh * 512 + off:(hh + 1) * 512],
                                kz[hh * 64:(hh + 1) * 64, :,
                                   kt * 128:(kt + 1) * 128],
                                qz[hh * 64:(hh + 1) * 64, :,
                                   qb * 512 + off:(qb + 1) * 512],
                                start=True, stop=True, perf_mode=DR,
                                tile_position=(hh * 64, 0))
                        sc_tiles[kt] = (sc, off)

                    pv = pp_pv.tile([65, 1024], f32, tag="pv")
                    emit_s(0)
                    for kt in range(nkt):
                        if kt + 1 < nkt:
                            emit_s(kt + 1)
                        sc, off = sc_tiles.pop(kt)
                        et = epool.tile([128, 1024], bf16, tag="exp")
                        et3 = et[:].rearrange("p (h c) -> p h c", h=2)
                        sc3 = sc[:].rearrange("p (h c) -> p h c", h=2)
                        nc.scalar.activation(
                            et3[:, :, off:512], sc3[:, :, off:512],
                            AF.Exp, scale=float(2.0 ** -13))
                        if kt - 4 * qb >= 0:
                            nc.vector.tensor_mul(
                                et3[:, :, off:off + 128],
                                et3[:, :, off:off + 128],
                                mask_sb[:].rearrange("p (h c) -> p h c", h=2))
                        for hh in range(2):
                            nc.tensor.matmul(
                                pv[:, hh * 512 + off:(hh + 1) * 512],
                                v_sb[:, kt * VSTR + (2 * pair + hh) * 65:
                                     kt * VSTR + (2 * pair + hh) * 65 + 65],
                                et[:, hh * 512 + off:(hh + 1) * 512],
                                start=(kt == 0), stop=(kt == nkt - 1))
                    den = rpool.tile([1, 1024], f32, tag="den")
                    nc.vector.tensor_copy(den[:], pv[64:65, :])
                    rc = rpool.tile([1, 1024], f32, tag="rc")
                    nc.vector.reciprocal_approx_fast(rc[:], den[:])
                    rb = rbpool.tile([64, 1024], f32, tag="rb")
                    nc.gpsimd.partition_broadcast(rb[:], rc[:])
                    for hh in range(2):
                        nc.vector.tensor_mul(
                            attn_sb[hh * 64:(hh + 1) * 64,
                                    pair * T + qb * 512: pair * T + qb * 512 + 512],
                            pv[0:64, hh * 512:(hh + 1) * 512],
                            rb[:, hh * 512:(hh + 1) * 512])

            # ---- output projection (bf16) ----
            for tb2 in range(NTB // 2):
                emit_outproj((2 * tb2, 2 * tb2 + 1), range(8))

    nc.compile()
    return nc


def _get_program():
    if "nc" not in _CACHE:
        _CACHE["nc"] = _build_program()
    return _CACHE["nc"]


def _make_core_inputs(x, w_qkv, b_qkv, w_out):
    f8 = ml_dtypes.float8_e4m3
    bf = ml_dtypes.bfloat16
    mask = np.triu(np.ones((128, 128), np.float32))
    mask2 = np.concatenate([mask, mask], axis=1).astype(bf)
    zf8 = np.zeros((128, T), f8)
    ins = []
    for c in range(8):
        b, hg = c // 2, c % 2
        h0 = hg * 512
        qsel = slice(h0, h0 + 512)
        ksel = slice(D + h0, D + h0 + 512)
        vsel = slice(2 * D + h0, 2 * D + h0 + 512)
        xT = np.ascontiguousarray(x[b].T)                       # [1024, T]
        xf8 = np.clip(xT * 32.0, -240, 240).astype(f8) \
            .reshape(KT, 128, T).transpose(1, 0, 2).reshape(128, KT * T)
        x16 = xT.astype(bf) \
            .reshape(KT, 128, T).transpose(1, 0, 2).reshape(128, KT * T)
        wqk = np.concatenate([w_qkv[qsel], w_qkv[ksel]], axis=0)  # [1024, D]
        wqk8 = np.clip(wqk * 2048.0, -240, 240).astype(f8) \
            .reshape(8, 128, 4, 2, 128).transpose(4, 0, 2, 3, 1) \
            .reshape(128, 8192)
        wv16 = np.ascontiguousarray(w_qkv[vsel].T).astype(bf) \
            .reshape(KT, 128, 512).transpose(1, 0, 2).reshape(128, KT * 512)
        bqk = 32.0 * np.concatenate([b_qkv[qsel], b_qkv[ksel]])
        ins.append({
            "xf8": np.ascontiguousarray(xf8),
            "x16": np.ascontiguousarray(x16),
            "wqk8": np.ascontiguousarray(wqk8),
            "wv16": np.ascontiguousarray(wv16),
            "bqk": np.ascontiguousarray(bqk.reshape(8, 128).T.astype(np.float32)),
            "bv": np.ascontiguousarray(
                np.broadcast_to(b_qkv[vsel], (128, 512)).astype(np.float32)),
            "zf8": zf8,
            "woT": np.ascontiguousarray(
                w_out[:, h0:h0 + 512].T).astype(bf),
            "mask2": mask2,
        })
    return ins


def kernel(x, w_qkv, b_qkv, w_out, b_out, _trace=False):
    from concourse.bass_utils import run_bass_kernel_spmd

    x = np.asarray(x, np.float32)
    w_qkv = np.asarray(w_qkv, np.float32)
    b_qkv = np.asarray(b_qkv, np.float32)
    w_out = np.asarray(w_out, np.float32)
    b_out = np.asarray(b_out, np.float32)

    nc = _get_program()
    ins = _make_core_inputs(x, w_qkv, b_qkv, w_out)
    res = run_bass_kernel_spmd(nc, ins, core_ids=list(range(8)), trace=_trace)
    _CACHE["last_result"] = res

    out = np.empty((B, T, D), np.float32)
    for b in range(B):
        s = res.results[2 * b]["outT"] + res.results[2 * b + 1]["outT"]
        out[b] = s.T + b_out
    return out


# revision 4
# speedup vs baseline: 1.0851x; 1.0851x over previous
"""Causal self-attention (B=4, T=2048, D=1024, H=16) on 8 trn2 NeuronCores.

Sharding: 2-D data x tensor parallel. Core c handles batch b = c//2 and
head group hg = c%2 (8 of the 16 heads). Each core computes its 8 heads'
qkv projection, causal attention, and a partial output projection
(columns of w_out for its heads); the host sums the two partials per
batch element and adds b_out.

All matmul paths are bf16 (same PE streaming rate as f32r, half the DMA
and SBUF): 512-col matmuls sustain ~216ns; wider fp8/DoubleRow modes draw
more power and trip the P0 downclock, netting a loss.
  - q/k projection -> PSUM f32; ACT Identity(scale=1, bias=b) converts to
    bf16 q/k stores [feat, T] while the vector engine stays free.
  - S^T tiles = matmul(lhsT=k[64,128], rhs=q[64,512]); two heads packed
    into PE rows 0-63 / 64-127 via tile_position. Diagonal blocks are
    narrowed to the live query range [off:512] for S, exp and PV, so the
    below-diagonal et region is never touched (no hot-loop memsets).
  - S(kt+1) is emitted before exp(kt)/PV(kt): PE streams the next score
    tile while ACT exponentiates the current one.
  - PV via v_aug [tok, 65] bf16 (65th col ones => PSUM row 64 = softmax
    denominator); DVE reciprocal + GpSimd partition-broadcast normalize.
  - out^T = w_outT-contract over attention features (bf16).
"""
import sys

import numpy as np

if "/opt/trn_rl_repo" not in sys.path:
    sys.path.insert(0, "/opt/trn_rl_repo")

import ml_dtypes

D = 1024          # d_model
T = 2048          # seq len
B = 4             # batch
HD = 64           # head dim
KT = 8            # d_model k-tiles of 128
NTT = 16          # token tiles of 128
NTB = 4           # token blocks of 512
NPAIR = 4         # head pairs per core (8 heads)
VSTR = 8 * 65     # v_aug cols per token tile (8 heads x 65)
SCALE = 1.0 / np.sqrt(HD)
WARMUP_MM = 32

_CACHE = {}


def _build_program():
    import concourse.mybir as mybir
    import concourse.tile as tile
    from concourse import bacc

    dt = mybir.dt
    f32, bf16 = dt.float32, dt.bfloat16
    AF = mybir.ActivationFunctionType

    nc = bacc.Bacc("TRN2", target_bir_lowering=False, debug=False,
                   enable_asserts=False, num_devices=8)

    x16_d = nc.dram_tensor("x16", [128, KT * T], bf16, kind="ExternalInput").ap()
    wqk_d = nc.dram_tensor("wqk16", [128, 8192], bf16, kind="ExternalInput").ap()
    wv16_d = nc.dram_tensor("wv16", [128, KT * 512], bf16, kind="ExternalInput").ap()
    bqk_d = nc.dram_tensor("bqk", [128, 8], f32, kind="ExternalInput").ap()
    bv_d = nc.dram_tensor("bv", [128, 512], f32, kind="ExternalInput").ap()
    woT_d = nc.dram_tensor("woT", [512, 1024], bf16, kind="ExternalInput").ap()
    mask_d = nc.dram_tensor("mask2", [128, 256], bf16, kind="ExternalInput").ap()
    outT_d = nc.dram_tensor("outT", [D, T], f32, kind="ExternalOutput").ap()
    warm_d = nc.dram_tensor("warm", [1, 512], f32, kind="ExternalOutput").ap()

    with tile.TileContext(nc) as tc:
        with tc.tile_pool(name="const", bufs=1) as cpool, \
             tc.tile_pool(name="qk16", bufs=1) as qkpool, \
             tc.tile_pool(name="xt", bufs=1) as xpool, \
             tc.tile_pool(name="vt", bufs=1) as vpool, \
             tc.tile_pool(name="exp", bufs=3) as epool, \
             tc.tile_pool(name="at", bufs=1) as apool, \
             tc.tile_pool(name="rcp", bufs=2) as rpool, \
             tc.tile_pool(name="rbc", bufs=2) as rbpool, \
             tc.tile_pool(name="stg", bufs=2) as spool, \
             tc.tile_pool(name="big", bufs=2, space="PSUM") as pp_big, \
             tc.tile_pool(name="pv", bufs=2, space="PSUM") as pp_pv:

            # ---- PE warm-up: keep the clock un-throttled during DMA ----
            wtile = cpool.tile([128, 512], bf16, tag="wrm")
            nc.vector.memset(wtile[:], 0.001)
            wps = pp_big.tile([128, 1024], f32, tag="big")
            for i in range(WARMUP_MM):
                nc.tensor.matmul(wps[:, 0:512], wtile[:, 0:128], wtile[:],
                                 start=(i == 0), stop=(i == WARMUP_MM - 1))
            wout = cpool.tile([1, 512], f32, tag="wout")
            nc.vector.tensor_copy(wout[:], wps[0:1, 0:512])
            nc.sync.dma_start(warm_d, wout[:])

            # ---- constant loads (ordered: first-needed first) ----
            bqk_sb = cpool.tile([128, 8], f32, tag="bqk")
            nc.sync.dma_start(bqk_sb[:], bqk_d)
            m_order = [0, 4, 1, 5, 2, 6, 3, 7]
            wqk_sb = cpool.tile([128, 8192], bf16, tag="wqk")
            for m in m_order:
                nc.sync.dma_start(wqk_sb[:, m * 1024:(m + 1) * 1024],
                                  wqk_d[:, m * 1024:(m + 1) * 1024])
            # x bf16 chunked [kt, half]; qk projection needs full T early
            x16_sb = xpool.tile([128, KT * T], bf16, tag="x16")
            for half in range(2):
                for kt in range(KT):
                    nc.sync.dma_start(
                        x16_sb[:, kt * T + half * 1024: kt * T + (half + 1) * 1024],
                        x16_d[:, kt * T + half * 1024: kt * T + (half + 1) * 1024])
            wv16_sb = cpool.tile([128, KT * 512], bf16, tag="wv")
            nc.sync.dma_start(wv16_sb[:], wv16_d)
            mask_sb = cpool.tile([128, 256], bf16, tag="mask")
            nc.sync.dma_start(mask_sb[:], mask_d)
            bv_sb = cpool.tile([128, 512], f32, tag="bv")
            nc.sync.dma_start(bv_sb[:], bv_d)
            woT_sb = cpool.tile([128, 4 * 1024], bf16, tag="wo")
            nc.sync.dma_start(
                woT_sb[:].rearrange("p (k f) -> p k f", k=4),
                woT_d.rearrange("(k p) f -> p k f", p=128))

            # ---- qk projection (bf16), all 8 feature blocks ----
            # qk16 store: [block m (8), T] bf16; ACT does psum->bf16 + bias
            qk16_sb = qkpool.tile([128, 8 * T], bf16, tag="qk16")
            for m in m_order:
                for nb2 in range(2):
                    ps = pp_big.tile([128, 1024], f32, tag="big")
                    for kt in range(KT):
                        for u in range(2):
                            c0 = nb2 * 1024 + u * 512
                            nc.tensor.matmul(
                                ps[:, u * 512:(u + 1) * 512],
                                wqk_sb[:, m * 1024 + kt * 128:
                                       m * 1024 + (kt + 1) * 128],
                                x16_sb[:, kt * T + c0: kt * T + c0 + 512],
                                start=(kt == 0), stop=(kt == KT - 1))
                    nc.scalar.activation(
                        qk16_sb[:, m * T + nb2 * 1024: m * T + nb2 * 1024 + 1024],
                        ps[:], AF.Identity,
                        bias=bqk_sb[:, m:m + 1], scale=1.0)

            # ---- v phase: v_aug [token, head*65] bf16 (65th col = ones) ----
            v_sb = vpool.tile([128, NTT * VSTR], bf16)
            nc.vector.memset(
                v_sb[:].rearrange("p (g e) -> p g e", e=65)[:, :, 64:65], 1.0)
            for tt2 in range(NTT // 2):
                ps = pp_big.tile([128, 1024], f32, tag="big")
                for kt in range(KT):
                    for u in range(2):
                        tt = 2 * tt2 + u
                        nc.tensor.matmul(
                            ps[:, u * 512:(u + 1) * 512],
                            x16_sb[:, kt * T + tt * 128: kt * T + (tt + 1) * 128],
                            wv16_sb[:, kt * 512:(kt + 1) * 512],
                            start=(kt == 0), stop=(kt == KT - 1))
                nc.vector.tensor_add(
                    v_sb[:].rearrange("p (t h e) -> p t h e", t=NTT, h=8)
                        [:, 2 * tt2:2 * tt2 + 2, :, 0:64],
                    ps[:].rearrange("p (u h f) -> p u h f", u=2, h=8),
                    bv_sb[:].rearrange("p (u h f) -> p u h f", u=1, h=8)
                         .to_broadcast([128, 2, 8, 64]))

            # ---- per head-pair attention ----
            attn_sb = apool.tile([128, NPAIR * T], bf16)

            def emit_outproj(tbs, mds):
                for md in mds:
                    ps = pp_big.tile([128, 1024], f32, tag="big")
                    for kf in range(4):
                        for u, tb in enumerate(tbs):
                            nc.tensor.matmul(
                                ps[:, u * 512:(u + 1) * 512],
                                woT_sb[:, kf * 1024 + md * 128:
                                       kf * 1024 + (md + 1) * 128],
                                attn_sb[:, kf * T + tb * 512:
                                        kf * T + tb * 512 + 512],
                                start=(kf == 0), stop=(kf == 3))
                    st = spool.tile([128, 1024], f32, tag="stg")
                    w = 512 * len(tbs)
                    nc.vector.tensor_copy(st[0:128, 0:w], ps[0:128, 0:w])
                    nc.sync.dma_start(
                        outT_d[md * 128:(md + 1) * 128,
                               tbs[0] * 512: tbs[0] * 512 + w], st[0:128, 0:w])

            for pair in range(NPAIR):
                q16 = qk16_sb[:, pair * T:(pair + 1) * T]
                k16 = qk16_sb[:, (4 + pair) * T:(5 + pair) * T]

                for qb in range(NTB):
                    nkt = 4 * qb + 4
                    sc_tiles = {}

                    def emit_s(kt, qb=qb, q16=q16, k16=k16, sc_tiles=sc_tiles):
                        off = max(kt - 4 * qb, 0) * 128
                        sc = pp_big.tile([128, 1024], f32, tag="big")
                        for hh in range(2):
                            nc.tensor.matmul(
                                sc[:, hh * 512 + off:(hh + 1) * 512],
                                k16[hh * 64:(hh + 1) * 64,
                                    kt * 128:(kt + 1) * 128],
                                q16[hh * 64:(hh + 1) * 64,
                                    qb * 512 + off:(qb + 1) * 512],
                                start=True, stop=True,
                                tile_position=(hh * 64, 0))
                        sc_tiles[kt] = (sc, off)

                    pv = pp_pv.tile([65, 1024], f32, tag="pv")
                    emit_s(0)
                    for kt in range(nkt):
                        if kt + 1 < nkt:
                            emit_s(kt + 1)
                        sc, off = sc_tiles.pop(kt)
                        et = epool.tile([128, 1024], bf16, tag="exp")
                        et3 = et[:].rearrange("p (h c) -> p h c", h=2)
                        sc3 = sc[:].rearrange("p (h c) -> p h c", h=2)
                        nc.scalar.activation(
                            et3[:, :, off:512], sc3[:, :, off:512],
                            AF.Exp, scale=float(SCALE))
                        if kt - 4 * qb >= 0:
                            nc.vector.tensor_mul(
                                et3[:, :, off:off + 128],
                                et3[:, :, off:off + 128],
                                mask_sb[:].rearrange("p (h c) -> p h c", h=2))
                        for hh in range(2):
                            nc.tensor.matmul(
                                pv[:, hh * 512 + off:(hh + 1) * 512],
                                v_sb[:, kt * VSTR + (2 * pair + hh) * 65:
                                     kt * VSTR + (2 * pair + hh) * 65 + 65],
                                et[:, hh * 512 + off:(hh + 1) * 512],
                                start=(kt == 0), stop=(kt == nkt - 1))
                    den = rpool.tile([1, 1024], f32, tag="den")
                    nc.vector.tensor_copy(den[:], pv[64:65, :])
                    rc = rpool.tile([1, 1024], f32, tag="rc")
                    nc.vector.reciprocal_approx_fast(rc[:], den[:])
                    rb = rbpool.tile([64, 1024], f32, tag="rb")
                    nc.gpsimd.partition_broadcast(rb[:], rc[:])
                    for hh in range(2):
                        nc.vector.tensor_mul(
                            attn_sb[hh * 64:(hh + 1) * 64,
                                    pair * T + qb * 512: pair * T + qb * 512 + 512],
                            pv[0:64, hh * 512:(hh + 1) * 512],
                            rb[:, hh * 512:(hh + 1) * 512])

            # ---- output projection (bf16) ----
            for tb2 in range(NTB // 2):
                emit_outproj((2 * tb2, 2 * tb2 + 1), range(8))

    nc.compile()
    return nc


def _get_program():
    if "nc" not in _CACHE:
        _CACHE["nc"] = _build_program()
    return _CACHE["nc"]


def _make_core_inputs(x, w_qkv, b_qkv, w_out):
    bf = ml_dtypes.bfloat16
    mask = np.triu(np.ones((128, 128), np.float32))
    mask2 = np.concatenate([mask, mask], axis=1).astype(bf)
    ins = []
    for c in range(8):
        b, hg = c // 2, c % 2
        h0 = hg * 512
        qsel = slice(h0, h0 + 512)
        ksel = slice(D + h0, D + h0 + 512)
        vsel = slice(2 * D + h0, 2 * D + h0 + 512)
        xT = np.ascontiguousarray(x[b].T)                       # [1024, T]
        x16 = xT.astype(bf) \
            .reshape(KT, 128, T).transpose(1, 0, 2).reshape(128, KT * T)
        wqk = np.concatenate([w_qkv[qsel], w_qkv[ksel]], axis=0)  # [1024, D]
        wqk16 = wqk.astype(bf) \
            .reshape(8, 128, 8, 128).transpose(3, 0, 2, 1).reshape(128, 8192)
        wv16 = np.ascontiguousarray(w_qkv[vsel].T).astype(bf) \
            .reshape(KT, 128, 512).transpose(1, 0, 2).reshape(128, KT * 512)
        bqk = np.concatenate([b_qkv[qsel], b_qkv[ksel]])
        ins.append({
            "x16": np.ascontiguousarray(x16),
            "wqk16": np.ascontiguousarray(wqk16),
            "wv16": np.ascontiguousarray(wv16),
            "bqk": np.ascontiguousarray(bqk.reshape(8, 128).T.astype(np.float32)),
            "bv": np.ascontiguousarray(
                np.broadcast_to(b_qkv[vsel], (128, 512)).astype(np.float32)),
            "woT": np.ascontiguousarray(
                w_out[:, h0:h0 + 512].T).astype(bf),
            "mask2": mask2,
        })
    return ins


def kernel(x, w_qkv, b_qkv, w_out, b_out, _trace=False):
    from concourse.bass_utils import run_bass_kernel_spmd

    x = np.asarray(x, np.float32)
    w_qkv = np.asarray(w_qkv, np.float32)
    b_qkv = np.asarray(b_qkv, np.float32)
    w_out = np.asarray(w_out, np.float32)
    b_out = np.asarray(b_out, np.float32)

    nc = _get_program()
    ins = _make_core_inputs(x, w_qkv, b_qkv, w_out)
    res = run_bass_kernel_spmd(nc, ins, core_ids=list(range(8)), trace=_trace)
    _CACHE["last_result"] = res

    out = np.empty((B, T, D), np.float32)
    for b in range(B):
        s = res.results[2 * b]["outT"] + res.results[2 * b + 1]["outT"]
        out[b] = s.T + b_out
    return out


# revision 9
# speedup vs baseline: 1.1123x; 1.0251x over previous
"""Causal self-attention (B=4, T=2048, D=1024, H=16) on 8 trn2 NeuronCores.

Sharding: 2-D data x tensor parallel. Core c handles batch b = c//2 and
head group hg = c%2 (8 of the 16 heads). Each core computes its 8 heads'
qkv projection, causal attention, and a partial output projection
(columns of w_out for its heads); the host sums the two partials per
batch element and adds b_out.

All matmul paths are bf16 (same PE streaming rate as f32r, half the DMA
and SBUF): 512-col matmuls sustain ~216ns; wider fp8/DoubleRow modes draw
more power and trip the P0 downclock, netting a loss.
  - q/k projection -> PSUM f32; ACT Identity(scale=1, bias=b) converts to
    bf16 q/k stores [feat, T] while the vector engine stays free.
  - S^T tiles = matmul(lhsT=k[64,128], rhs=q[64,512]); two heads packed
    into PE rows 0-63 / 64-127 via tile_position. Diagonal blocks are
    narrowed to the live query range [off:512] for S, exp and PV, so the
    below-diagonal et region is never touched (no hot-loop memsets).
  - S(kt+1) is emitted before exp(kt)/PV(kt): PE streams the next score
    tile while ACT exponentiates the current one.
  - PV via v_aug [tok, 65] bf16 (65th col ones => PSUM row 64 = softmax
    denominator); DVE reciprocal + GpSimd partition-broadcast normalize.
  - out^T = w_outT-contract over attention features (bf16).
"""
import sys

import numpy as np

if "/opt/trn_rl_repo" not in sys.path:
    sys.path.insert(0, "/opt/trn_rl_repo")

import ml_dtypes

D = 1024          # d_model
T = 2048          # seq len
B = 4             # batch
HD = 64           # head dim
KT = 8            # d_model k-tiles of 128
NTT = 16          # token tiles of 128
NTB = 4           # token blocks of 512
NPAIR = 4         # head pairs per core (8 heads)
VSTR = 8 * 65     # v_aug cols per token tile (8 heads x 65)
SCALE = 1.0 / np.sqrt(HD)
WARMUP_MM = 32

_CACHE = {}


def _build_program():
    import concourse.mybir as mybir
    import concourse.tile as tile
    from concourse import bacc

    dt = mybir.dt
    f32, bf16 = dt.float32, dt.bfloat16
    AF = mybir.ActivationFunctionType

    nc = bacc.Bacc("TRN2", target_bir_lowering=False, debug=False,
                   enable_asserts=False, num_devices=8)

    x16_d = nc.dram_tensor("x16", [128, KT * T], bf16, kind="ExternalInput").ap()
    wqk_d = nc.dram_tensor("wqk16", [128, 8192], bf16, kind="ExternalInput").ap()
    wv16_d = nc.dram_tensor("wv16", [128, KT * 512], bf16, kind="ExternalInput").ap()
    bqk_d = nc.dram_tensor("bqk", [128, 8], f32, kind="ExternalInput").ap()
    bv_d = nc.dram_tensor("bv", [128, 512], f32, kind="ExternalInput").ap()
    woT_d = nc.dram_tensor("woT", [512, 1024], bf16, kind="ExternalInput").ap()
    mask_d = nc.dram_tensor("mask2", [128, 256], bf16, kind="ExternalInput").ap()
    outT_d = nc.dram_tensor("outT", [D, T], bf16, kind="ExternalOutput").ap()
    warm_d = nc.dram_tensor("warm", [1, 512], f32, kind="ExternalOutput").ap()

    with tile.TileContext(nc) as tc:
        with tc.tile_pool(name="const", bufs=1) as cpool, \
             tc.tile_pool(name="qk16", bufs=1) as qkpool, \
             tc.tile_pool(name="xt", bufs=1) as xpool, \
             tc.tile_pool(name="vt", bufs=1) as vpool, \
             tc.tile_pool(name="exp", bufs=3) as epool, \
             tc.tile_pool(name="at", bufs=1) as apool, \
             tc.tile_pool(name="rcp", bufs=2) as rpool, \
             tc.tile_pool(name="rbc", bufs=2) as rbpool, \
             tc.tile_pool(name="stg", bufs=2) as spool, \
             tc.tile_pool(name="big", bufs=2, space="PSUM") as pp_big, \
             tc.tile_pool(name="pv", bufs=2, space="PSUM") as pp_pv:

            # ---- PE warm-up: keep the clock un-throttled during DMA ----
            wtile = cpool.tile([128, 512], bf16, tag="wrm")
            nc.vector.memset(wtile[:], 0.001)
            wps = pp_big.tile([128, 1024], f32, tag="big")
            for i in range(WARMUP_MM):
                nc.tensor.matmul(wps[:, 0:512], wtile[:, 0:128], wtile[:],
                                 start=(i == 0), stop=(i == WARMUP_MM - 1))
            wout = cpool.tile([1, 512], f32, tag="wout")
            nc.vector.tensor_copy(wout[:], wps[0:1, 0:512])
            nc.sync.dma_start(warm_d, wout[:])

            # ---- constant loads (ordered: first-needed first) ----
            bqk_sb = cpool.tile([128, 8], f32, tag="bqk")
            nc.sync.dma_start(bqk_sb[:], bqk_d)
            m_order = [0, 4, 1, 5, 2, 6, 3, 7]
            wqk_sb = cpool.tile([128, 8192], bf16, tag="wqk")
            for m in m_order:
                nc.sync.dma_start(wqk_sb[:, m * 1024:(m + 1) * 1024],
                                  wqk_d[:, m * 1024:(m + 1) * 1024])
            # x bf16 chunked [kt, half]; qk projection needs full T early
            x16_sb = xpool.tile([128, KT * T], bf16, tag="x16")
            for half in range(2):
                for kt in range(KT):
                    nc.sync.dma_start(
                        x16_sb[:, kt * T + half * 1024: kt * T + (half + 1) * 1024],
                        x16_d[:, kt * T + half * 1024: kt * T + (half + 1) * 1024])
            wv16_sb = cpool.tile([128, KT * 512], bf16, tag="wv")
            nc.sync.dma_start(wv16_sb[:], wv16_d)
            mask_sb = cpool.tile([128, 256], bf16, tag="mask")
            nc.sync.dma_start(mask_sb[:], mask_d)
            bv_sb = cpool.tile([128, 512], f32, tag="bv")
            nc.sync.dma_start(bv_sb[:], bv_d)
            woT_sb = cpool.tile([128, 4 * 1024], bf16, tag="wo")
            nc.sync.dma_start(
                woT_sb[:].rearrange("p (k f) -> p k f", k=4),
                woT_d.rearrange("(k p) f -> p k f", p=128))

            # ---- qk projection (bf16), all 8 feature blocks ----
            # qk16 store: [block m (8), T] bf16; ACT does psum->bf16 + bias
            qk16_sb = qkpool.tile([128, 8 * T], bf16, tag="qk16")
            for m in m_order:
                for nb2 in range(2):
                    ps = pp_big.tile([128, 1024], f32, tag="big")
                    for kt in range(KT):
                        for u in range(2):
                            c0 = nb2 * 1024 + u * 512
                            nc.tensor.matmul(
                                ps[:, u * 512:(u + 1) * 512],
                                wqk_sb[:, m * 1024 + kt * 128:
                                       m * 1024 + (kt + 1) * 128],
                                x16_sb[:, kt * T + c0: kt * T + c0 + 512],
                                start=(kt == 0), stop=(kt == KT - 1))
                    nc.scalar.activation(
                        qk16_sb[:, m * T + nb2 * 1024: m * T + nb2 * 1024 + 1024],
                        ps[:], AF.Identity,
                        bias=bqk_sb[:, m:m + 1], scale=1.0)

            # ---- v phase: v_aug [token, head*65] bf16 (65th col = ones) ----
            v_sb = vpool.tile([128, NTT * VSTR], bf16)
            nc.vector.memset(
                v_sb[:].rearrange("p (g e) -> p g e", e=65)[:, :, 64:65], 1.0)
            for tt2 in range(NTT // 2):
                ps = pp_big.tile([128, 1024], f32, tag="big")
                for kt in range(KT):
                    for u in range(2):
                        tt = 2 * tt2 + u
                        nc.tensor.matmul(
                            ps[:, u * 512:(u + 1) * 512],
                            x16_sb[:, kt * T + tt * 128: kt * T + (tt + 1) * 128],
                            wv16_sb[:, kt * 512:(kt + 1) * 512],
                            start=(kt == 0), stop=(kt == KT - 1))
                nc.vector.tensor_add(
                    v_sb[:].rearrange("p (t h e) -> p t h e", t=NTT, h=8)
                        [:, 2 * tt2:2 * tt2 + 2, :, 0:64],
                    ps[:].rearrange("p (u h f) -> p u h f", u=2, h=8),
                    bv_sb[:].rearrange("p (u h f) -> p u h f", u=1, h=8)
                         .to_broadcast([128, 2, 8, 64]))

            # ---- per head-pair attention ----
            attn_sb = apool.tile([128, NPAIR * T], bf16)

            def emit_outproj(tbs, mds):
                for md in mds:
                    ps = pp_big.tile([128, 1024], f32, tag="big")
                    for kf in range(4):
                        for u, tb in enumerate(tbs):
                            nc.tensor.matmul(
                                ps[:, u * 512:(u + 1) * 512],
                                woT_sb[:, kf * 1024 + md * 128:
                                       kf * 1024 + (md + 1) * 128],
                                attn_sb[:, kf * T + tb * 512:
                                        kf * T + tb * 512 + 512],
                                start=(kf == 0), stop=(kf == 3))
                    st = spool.tile([128, 1024], bf16, tag="stg")
                    w = 512 * len(tbs)
                    nc.scalar.activation(st[0:128, 0:w], ps[0:128, 0:w],
                                         AF.Identity, bias=0.0, scale=1.0)
                    nc.sync.dma_start(
                        outT_d[md * 128:(md + 1) * 128,
                               tbs[0] * 512: tbs[0] * 512 + w], st[0:128, 0:w])

            # flat (pair, qb, kt) stream: the S 1-ahead crosses qb and pair
            # boundaries so the PE<->ACT pipeline never drains
            seq = [(pair, qb, kt)
                   for pair in range(NPAIR)
                   for qb in range(NTB)
                   for kt in range(4 * qb + 4)]
            sc_tiles = {}
            pv_tiles = {}

            def emit_s(pair, qb, kt):
                off = max(kt - 4 * qb, 0) * 128
                sc = pp_big.tile([128, 1024], f32, tag="big")
                for hh in range(2):
                    nc.tensor.matmul(
                        sc[:, hh * 512 + off:(hh + 1) * 512],
                        qk16_sb[(hh * 64):(hh + 1) * 64,
                                (4 + pair) * T + kt * 128:
                                (4 + pair) * T + (kt + 1) * 128],
                        qk16_sb[(hh * 64):(hh + 1) * 64,
                                pair * T + qb * 512 + off:
                                pair * T + (qb + 1) * 512],
                        start=True, stop=True,
                        tile_position=(hh * 64, 0))
                sc_tiles[(pair, qb, kt)] = (sc, off)

            emit_s(*seq[0])
            for i, (pair, qb, kt) in enumerate(seq):
                nkt = 4 * qb + 4
                if i + 1 < len(seq):
                    emit_s(*seq[i + 1])
                if kt == 0:
                    pv_tiles[(pair, qb)] = pp_pv.tile(
                        [65, 1024], f32, tag="pv", name="pv")
                pv = pv_tiles[(pair, qb)]
                sc, off = sc_tiles.pop((pair, qb, kt))
                et = epool.tile([128, 1024], bf16, tag="exp")
                et3 = et[:].rearrange("p (h c) -> p h c", h=2)
                sc3 = sc[:].rearrange("p (h c) -> p h c", h=2)
                nc.scalar.activation(
                    et3[:, :, off:512], sc3[:, :, off:512],
                    AF.Exp, scale=float(SCALE))
                if kt - 4 * qb >= 0:
                    nc.vector.tensor_mul(
                        et3[:, :, off:off + 128],
                        et3[:, :, off:off + 128],
                        mask_sb[:].rearrange("p (h c) -> p h c", h=2))
                for hh in range(2):
                    nc.tensor.matmul(
                        pv[:, hh * 512 + off:(hh + 1) * 512],
                        v_sb[:, kt * VSTR + (2 * pair + hh) * 65:
                             kt * VSTR + (2 * pair + hh) * 65 + 65],
                        et[:, hh * 512 + off:(hh + 1) * 512],
                        start=(kt == 0), stop=(kt == nkt - 1))
                if kt == nkt - 1:
                    pv_tiles.pop((pair, qb))
                    den = rpool.tile([1, 1024], f32, tag="den")
                    nc.vector.tensor_copy(den[:], pv[64:65, :])
                    rc = rpool.tile([1, 1024], f32, tag="rc")
                    nc.vector.reciprocal_approx_fast(rc[:], den[:])
                    rb = rbpool.tile([64, 1024], f32, tag="rb")
                    nc.gpsimd.partition_broadcast(rb[:], rc[:])
                    for hh in range(2):
                        nc.vector.tensor_mul(
                            attn_sb[hh * 64:(hh + 1) * 64,
                                    pair * T + qb * 512:
                                    pair * T + qb * 512 + 512],
                            pv[0:64, hh * 512:(hh + 1) * 512],
                            rb[:, hh * 512:(hh + 1) * 512])

            # ---- output projection (bf16) ----
            for tb2 in range(NTB // 2):
                emit_outproj((2 * tb2, 2 * tb2 + 1), range(8))

    nc.compile()
    return nc


def _get_program():
    if "nc" not in _CACHE:
        _CACHE["nc"] = _build_program()
    return _CACHE["nc"]


def _make_core_inputs(x, w_qkv, b_qkv, w_out):
    bf = ml_dtypes.bfloat16
    mask = np.triu(np.ones((128, 128), np.float32))
    mask2 = np.concatenate([mask, mask], axis=1).astype(bf)
    ins = []
    for c in range(8):
        b, hg = c // 2, c % 2
        h0 = hg * 512
        qsel = slice(h0, h0 + 512)
        ksel = slice(D + h0, D + h0 + 512)
        vsel = slice(2 * D + h0, 2 * D + h0 + 512)
        xT = np.ascontiguousarray(x[b].T)                       # [1024, T]
        x16 = xT.astype(bf) \
            .reshape(KT, 128, T).transpose(1, 0, 2).reshape(128, KT * T)
        wqk = np.concatenate([w_qkv[qsel], w_qkv[ksel]], axis=0)  # [1024, D]
        wqk16 = wqk.astype(bf) \
            .reshape(8, 128, 8, 128).transpose(3, 0, 2, 1).reshape(128, 8192)
        wv16 = np.ascontiguousarray(w_qkv[vsel].T).astype(bf) \
            .reshape(KT, 128, 512).transpose(1, 0, 2).reshape(128, KT * 512)
        bqk = np.concatenate([b_qkv[qsel], b_qkv[ksel]])
        ins.append({
            "x16": np.ascontiguousarray(x16),
            "wqk16": np.ascontiguousarray(wqk16),
            "wv16": np.ascontiguousarray(wv16),
            "bqk": np.ascontiguousarray(bqk.reshape(8, 128).T.astype(np.float32)),
            "bv": np.ascontiguousarray(
                np.broadcast_to(b_qkv[vsel], (128, 512)).astype(np.float32)),
            "woT": np.ascontiguousarray(
                w_out[:, h0:h0 + 512].T).astype(bf),
            "mask2": mask2,
        })
    return ins


def kernel(x, w_qkv, b_qkv, w_out, b_out, _trace=False):
    from concourse.bass_utils import run_bass_kernel_spmd

    x = np.asarray(x, np.float32)
    w_qkv = np.asarray(w_qkv, np.float32)
    b_qkv = np.asarray(b_qkv, np.float32)
    w_out = np.asarray(w_out, np.float32)
    b_out = np.asarray(b_out, np.float32)

    nc = _get_program()
    ins = _make_core_inputs(x, w_qkv, b_qkv, w_out)
    res = run_bass_kernel_spmd(nc, ins, core_ids=list(range(8)), trace=_trace)
    _CACHE["last_result"] = res

    out = np.empty((B, T, D), np.float32)
    for b in range(B):
        s = res.results[2 * b]["outT"].astype(np.float32) \
            + res.results[2 * b + 1]["outT"].astype(np.float32)
        out[b] = s.T + b_out
    return out
